# revision 19
# baseline (speedup 1.0000x reference)
"""Trainium2 Bass kernel for CombinedSARAFilter.

Math: with D_t = I_t - I_{t-1} (I_{-1}=0), the module reduces to
    x_t = lam_r x_{t-1} + p D_t + q I_t
    o_t = lam_d o_{t-1} + a_d x_t + c3 |D_t|        (out = o, since TAU_RA == TAU_D)

Blocked linear scan, time chunks of L=125 on partitions; the 2e-2 tolerance
lets I/O compress: fp8(e4m3) input feeds the matmuls directly, bf16 output.
Per 512-lane block: 3 matmuls — WD (differences, for the |D| nonlinearity),
W_IN (input response, fp8), WA_AUG (abs response + carry response, bf16).
The (x, o-MU) carry rides rows 0-1 of the bf16 a-tile: the abs activation
zeroes them (WD's cols 0-1 are zero), then a cheap SBUF copy from the
previous chunk's bf16 out_t overwrites them — emitted after the abs so the
write order is right. DMA: input on sync, output alternating scalar/gpsimd
(per-queue DMA caps at ~120 GB/s, so each stream gets its own queue(s)).
"""
import sys

sys.path.insert(0, "/opt/trn_rl_repo")

import numpy as np
import ml_dtypes

BF16 = ml_dtypes.bfloat16
FP8 = ml_dtypes.float8_e4m3
X_DT = FP8

# filter constants
DT = 0.1
TAU_RA, K3 = 30.0, 2.0
TAU_R, TAU_D, K1, K2 = 5.0, 30.0, 0.05, 3.0
A_R = DT / TAU_R
A_D = DT / TAU_D
LAM_R = 1.0 - A_R
LAM_D = 1.0 - A_D
P = A_R * K2 / DT
Q = A_R * K1
C3 = K3 / TAU_RA
MU = 22.5  # mean shift: carry o-MU so the bf16 state stays small

B, T, N = 8, 2000, 2048
L = 125            # time chunk (on partitions)
NCH = T // L       # 16
NB = 512           # lane block (PSUM bank = 512 fp32)
NBLK = N // NB     # 4


def build_weights():
    """Host-side fp64 construction of the chunk filter matrices."""
    i = np.arange(L)
    Mr = np.tril(LAM_R ** (i[:, None] - i[None, :]))
    Md = np.tril(LAM_D ** (i[:, None] - i[None, :]))
    Bp = np.zeros((L, L + 1))
    Bp[i, i + 1] = 1.0
    Bp[i, i] = -1.0
    U = P * Bp
    U[:, 1:] += Q * np.eye(L)
    F1 = A_D * Md @ Mr @ U                  # [125 out, 126 in] o response to ihat
    v1 = LAM_D ** (i + 1)                   # o response to o'_in
    v2 = A_D * (Md @ (LAM_R ** (i + 1)))    # o response to x_in
    ones_resp = -A_D * MU * Md.sum(1)       # response of the constant -a_d*MU
    xrow_I = (Mr @ U)[L - 1]

    # out cols everywhere: 0=x_out, 1=o'_out, 2..126 = y_0..y_124

    # W_IN (lhsT) [126, 127] fp8: response to ihat rows [I_prev, I_0..I_124]
    W_in = np.zeros((126, 127))
    W_in[:, 0] = xrow_I
    W_in[:, 1] = F1[L - 1]
    W_in[:, 2:] = F1.T

    # WD (lhsT) [126, 127]: cols 0-1 zero (carry slots), col 2+t = D_t
    WD = np.zeros((126, 127))
    WD[i + 1, i + 2] = 1.0
    WD[i, i + 2] -= 1.0

    # WA_AUG (lhsT) [127, 127] bf16: rows 0-1 = carry response, rows 2.. = Md
    WA = np.zeros((127, 127))
    WA[0, 0] = LAM_R ** L
    WA[0, 1] = v2[L - 1]
    WA[0, 2:] = v2
    WA[1, 1] = v1[L - 1]
    WA[1, 2:] = v1
    WA[2:, 1] = Md[L - 1]
    WA[2:, 2:] = Md.T

    # bias column: x_out 0, o'_out ones_resp[-1], y rows ones_resp + MU
    bias = np.zeros((127, 1))
    bias[1, 0] = ones_resp[L - 1]
    bias[2:, 0] = ones_resp + MU

    s0 = np.zeros((2, N))
    s0[1] = -MU                      # initial carry (x=0, o'=-MU)
    z8 = np.zeros((1, N))            # chunk-0 I_prev row

    return {
        "W_IN": W_in.astype(FP8),
        "WD": WD.astype(FP8),
        "WA": WA.astype(BF16),
        "BIAS": bias.astype(np.float32),
        "S0": s0.astype(BF16),
        "Z8": z8.astype(FP8),
    }


_WDT = {"W_IN": "float8e4", "WD": "float8e4", "WA": "bfloat16",
        "BIAS": "float32", "S0": "bfloat16", "Z8": "float8e4"}


def build_program(reps: int = 1, mode: str = "full"):
    """Emit the single-core SPMD program. Returns (nc, weight_arrays)."""
    from concourse import bacc, mybir, tile

    dt = mybir.dt
    w = build_weights()

    nc = bacc.Bacc("TRN2", target_bir_lowering=False, debug=False)

    X = nc.dram_tensor("X", [T, N], dt.float8e4, kind="ExternalInput")
    Y = nc.dram_tensor("Y", [T, N], dt.bfloat16, kind="ExternalOutput")
    wd = {
        name: nc.dram_tensor(name, list(arr.shape), getattr(dt, _WDT[name]),
                             kind="ExternalInput")
        for name, arr in w.items()
    }

    with tile.TileContext(nc) as tc:
        with (
            tc.tile_pool(name="wpool", bufs=1) as wpool,
            tc.tile_pool(name="ipool", bufs=3) as ipool,
            tc.tile_pool(name="opool", bufs=3) as opool,
            tc.tile_pool(name="apool", bufs=4) as apool,
            tc.tile_pool(name="psD", bufs=2, space="PSUM") as psD,
            tc.tile_pool(name="psO", bufs=3, space="PSUM") as psO,
        ):
            wt = {}
            for name, arr in w.items():
                t_ = wpool.tile(list(arr.shape), getattr(dt, _WDT[name]), tag=name)
                nc.sync.dma_start(out=t_[:], in_=wd[name][:])
                wt[name] = t_

            out_engs = [nc.scalar, nc.gpsimd]
            for rep in range(reps):
                ihat = ipool.tile([126, N], dt.float8e4, tag="ihat")
                nc.sync.dma_start(out=ihat[0:1, :], in_=wd["Z8"][:])
                nc.sync.dma_start(out=ihat[1:126, :], in_=X[0:L, :])
                prev_out = None

                for k in range(NCH):
                    if k + 1 < NCH:
                        ihat_next = ipool.tile([126, N], dt.float8e4, tag="ihat")
                        nc.sync.dma_start(
                            out=ihat_next[:, :],
                            in_=X[(k + 1) * L - 1:(k + 2) * L, :],
                        )
                    else:
                        ihat_next = None

                    out_t = opool.tile([127, N], dt.bfloat16, tag="out")
                    if mode == "dma":
                        for half in range(2):
                            c0 = half * 2 * NB
                            nc.vector.tensor_copy(
                                out_t[0:125, c0:c0 + 2 * NB],
                                ihat[1:126, c0:c0 + 2 * NB].bitcast(dt.int8),
                            )
                        out_engs[k % 2].dma_start(
                            out=Y[k * L:(k + 1) * L, :], in_=out_t[0:125, :]
                        )
                        ihat = ihat_next
                        continue

                    for pair in range(2):
                        p0 = pair * 2 * NB
                        a2 = apool.tile([127, 2 * NB], dt.bfloat16, tag="A")
                        op = psO.tile([127, 2 * NB], dt.float32, tag="O")
                        d_ps = []
                        for half in range(2):
                            dp = psD.tile([127, NB], dt.float32, tag="D")
                            nc.tensor.matmul(
                                dp[:], wt["WD"][:],
                                ihat[:, p0 + half * NB:p0 + (half + 1) * NB],
                                start=True, stop=True,
                            )
                            d_ps.append(dp)
                        for half in range(2):
                            # abs zeroes rows 0-1 (WD cols 0-1 are zero)...
                            nc.scalar.activation(
                                a2[:, half * NB:(half + 1) * NB], d_ps[half][:],
                                func=mybir.ActivationFunctionType.Abs,
                                scale=float(C3),
                            )
                        # ...then the carry overwrites them (bf16, from the
                        # previous chunk's out_t rows 0-1; S0 for chunk 0)
                        if prev_out is None:
                            nc.sync.dma_start(
                                out=a2[0:2, :], in_=wd["S0"][:, p0:p0 + 2 * NB]
                            )
                        else:
                            nc.vector.tensor_copy(
                                a2[0:2, :], prev_out[0:2, p0:p0 + 2 * NB]
                            )
                        for half in range(2):
                            c0 = p0 + half * NB
                            col = half * NB
                            nc.tensor.matmul(
                                op[:, col:col + NB], wt["W_IN"][:],
                                ihat[:, c0:c0 + NB], start=True, stop=False,
                            )
                            nc.tensor.matmul(
                                op[:, col:col + NB], wt["WA"][:],
                                a2[:, col:col + NB], start=False, stop=True,
                            )
                        # PSUM -> SBUF with bias, right behind this pair's
                        # matmuls (shortens the carry tail); all on DVE — ACT
                        # is saturated by the four abs
                        nc.vector.tensor_scalar_add(
                            out_t[:, p0:p0 + 2 * NB], op[:, :],
                            wt["BIAS"][:, 0:1],
                        )
                    # emit chunk k-1's out-DMA here: its wait is satisfied by
                    # now, so the issuing sequencer (ACT for scalar) never
                    # blocks mid-chunk
                    if k > 0:
                        out_engs[(k - 1) % 2].dma_start(
                            out=Y[(k - 1) * L:k * L, :], in_=pend_out[2:127, :]
                        )
                    pend_out = out_t
                    prev_out = out_t
                    ihat = ihat_next
                out_engs[(NCH - 1) % 2].dma_start(
                    out=Y[(NCH - 1) * L:NCH * L, :], in_=pend_out[2:127, :]
                )

    nc.compile()
    return nc, w


_PROGRAM_CACHE = {}


def _get_program():
    if "nc" not in _PROGRAM_CACHE:
        nc, w = build_program()
        _PROGRAM_CACHE["nc"] = nc
        _PROGRAM_CACHE["w"] = w
    return _PROGRAM_CACHE["nc"], _PROGRAM_CACHE["w"]


def kernel(I_in: np.ndarray) -> np.ndarray:
    """Full-input entry point: I_in [8, 2000, 2048] fp32 -> out same shape."""
    from concourse.bass_utils import run_bass_kernel_spmd

    nc, w = _get_program()
    Xq = np.ascontiguousarray(I_in, dtype=np.float32).astype(X_DT)
    in_maps = [{"X": Xq[b], **w} for b in range(B)]
    last_err = None
    for _attempt in range(3):
        try:
            res = run_bass_kernel_spmd(nc, in_maps, list(range(B)))
            return np.stack(
                [res.results[b]["Y"].astype(np.float32) for b in range(B)], axis=0
            )
        except Exception as e:  # transient device errors: retry
            last_err = e
            import time as _time
            _time.sleep(5)
    raise last_err


if __name__ == "__main__":
    rng = np.random.default_rng(0)
    I = rng.standard_normal((B, T, N), dtype=np.float32)
    out = kernel(I)
    print(out.shape, out.dtype, np.abs(out).max())


# revision 21
# speedup vs baseline: 1.1101x; 1.1101x over previous
"""Trainium2 Bass kernel for CombinedSARAFilter.

Math: with D_t = I_t - I_{t-1} (I_{-1}=0), the module reduces to
    x_t = lam_r x_{t-1} + p D_t + q I_t
    o_t = lam_d o_{t-1} + a_d x_t + c3 |D_t|        (out = o, since TAU_RA == TAU_D)

Blocked linear scan, time chunks of L=125 on partitions; the 2e-2 tolerance
lets I/O compress: fp8(e4m3) input feeds the matmuls directly, bf16 output.
Per 512-lane block: 3 matmuls — WD (differences, for the |D| nonlinearity),
W_IN (input response, fp8), WA_AUG (abs response + carry response, bf16).
The (x, o-MU) carry rides rows 0-1 of the bf16 a-tile: the abs activation
zeroes them (WD's cols 0-1 are zero), then a cheap SBUF copy from the
previous chunk's bf16 out_t overwrites them — emitted after the abs so the
write order is right. DMA: input on sync, output alternating scalar/gpsimd
(per-queue DMA caps at ~120 GB/s, so each stream gets its own queue(s)).
"""
import sys

sys.path.insert(0, "/opt/trn_rl_repo")

import numpy as np
import ml_dtypes

BF16 = ml_dtypes.bfloat16
FP8 = ml_dtypes.float8_e4m3
X_DT = FP8

# filter constants
DT = 0.1
TAU_RA, K3 = 30.0, 2.0
TAU_R, TAU_D, K1, K2 = 5.0, 30.0, 0.05, 3.0
A_R = DT / TAU_R
A_D = DT / TAU_D
LAM_R = 1.0 - A_R
LAM_D = 1.0 - A_D
P = A_R * K2 / DT
Q = A_R * K1
C3 = K3 / TAU_RA
MU = 22.5  # mean shift: carry o-MU so the bf16 state stays small

B, T, N = 8, 2000, 2048
L = 125            # time chunk (on partitions)
NCH = T // L       # 16
NB = 512           # lane block (PSUM bank = 512 fp32)
NBLK = N // NB     # 4


def build_weights():
    """Host-side fp64 construction of the chunk filter matrices."""
    i = np.arange(L)
    Mr = np.tril(LAM_R ** (i[:, None] - i[None, :]))
    Md = np.tril(LAM_D ** (i[:, None] - i[None, :]))
    Bp = np.zeros((L, L + 1))
    Bp[i, i + 1] = 1.0
    Bp[i, i] = -1.0
    U = P * Bp
    U[:, 1:] += Q * np.eye(L)
    F1 = A_D * Md @ Mr @ U                  # [125 out, 126 in] o response to ihat
    v1 = LAM_D ** (i + 1)                   # o response to o'_in
    v2 = A_D * (Md @ (LAM_R ** (i + 1)))    # o response to x_in
    ones_resp = -A_D * MU * Md.sum(1)       # response of the constant -a_d*MU
    xrow_I = (Mr @ U)[L - 1]

    # out cols everywhere: 0=x_out, 1=o'_out, 2..126 = y_0..y_124

    # W_IN (lhsT) [126, 127] fp8: response to ihat rows [I_prev, I_0..I_124]
    W_in = np.zeros((126, 127))
    W_in[:, 0] = xrow_I
    W_in[:, 1] = F1[L - 1]
    W_in[:, 2:] = F1.T

    # WD (lhsT) [126, 127]: cols 0-1 zero (carry slots), col 2+t = D_t
    WD = np.zeros((126, 127))
    WD[i + 1, i + 2] = 1.0
    WD[i, i + 2] -= 1.0

    # WA_AUG (lhsT) [127, 127] bf16: rows 0-1 = carry response, rows 2.. = Md
    WA = np.zeros((127, 127))
    WA[0, 0] = LAM_R ** L
    WA[0, 1] = v2[L - 1]
    WA[0, 2:] = v2
    WA[1, 1] = v1[L - 1]
    WA[1, 2:] = v1
    WA[2:, 1] = Md[L - 1]
    WA[2:, 2:] = Md.T

    # bias column: x_out 0, o'_out ones_resp[-1], y rows ones_resp + MU
    bias = np.zeros((127, 1))
    bias[1, 0] = ones_resp[L - 1]
    bias[2:, 0] = ones_resp + MU

    s0 = np.zeros((2, N))
    s0[1] = -MU                      # initial carry (x=0, o'=-MU)
    z8 = np.zeros((1, N))            # chunk-0 I_prev row

    return {
        "W_IN": W_in.astype(FP8),
        "WD": WD.astype(FP8),
        "WA": WA.astype(BF16),
        "BIAS": bias.astype(np.float32),
        "S0": s0.astype(BF16),
        "Z8": z8.astype(FP8),
    }


_WDT = {"W_IN": "float8e4", "WD": "float8e4", "WA": "bfloat16",
        "BIAS": "float32", "S0": "bfloat16", "Z8": "float8e4"}


def build_program(reps: int = 1, mode: str = "full"):
    """Emit the single-core SPMD program. Returns (nc, weight_arrays)."""
    from concourse import bacc, mybir, tile

    dt = mybir.dt
    w = build_weights()

    nc = bacc.Bacc("TRN2", target_bir_lowering=False, debug=False)

    X = nc.dram_tensor("X", [T, N], dt.float8e4, kind="ExternalInput")
    Y = nc.dram_tensor("Y", [T, N], dt.bfloat16, kind="ExternalOutput")
    wd = {
        name: nc.dram_tensor(name, list(arr.shape), getattr(dt, _WDT[name]),
                             kind="ExternalInput")
        for name, arr in w.items()
    }

    with tile.TileContext(nc) as tc:
        with (
            tc.tile_pool(name="wpool", bufs=1) as wpool,
            tc.tile_pool(name="ipool", bufs=3) as ipool,
            tc.tile_pool(name="opool", bufs=3) as opool,
            tc.tile_pool(name="apool", bufs=4) as apool,
            tc.tile_pool(name="psD", bufs=2, space="PSUM") as psD,
            tc.tile_pool(name="psO", bufs=3, space="PSUM") as psO,
        ):
            wt = {}
            for name, arr in w.items():
                t_ = wpool.tile(list(arr.shape), getattr(dt, _WDT[name]), tag=name)
                nc.sync.dma_start(out=t_[:], in_=wd[name][:])
                wt[name] = t_

            out_engs = [nc.scalar, nc.gpsimd]
            for rep in range(reps):
                ihat = ipool.tile([126, N], dt.float8e4, tag="ihat")
                nc.sync.dma_start(out=ihat[0:1, :], in_=wd["Z8"][:])
                nc.sync.dma_start(out=ihat[1:126, :], in_=X[0:L, :])
                prev_out = None

                for k in range(NCH):
                    if k + 1 < NCH:
                        ihat_next = ipool.tile([126, N], dt.float8e4, tag="ihat")
                        nc.sync.dma_start(
                            out=ihat_next[:, :],
                            in_=X[(k + 1) * L - 1:(k + 2) * L, :],
                        )
                    else:
                        ihat_next = None

                    out_t = opool.tile([127, N], dt.bfloat16, tag="out")
                    if mode == "dma":
                        for half in range(2):
                            c0 = half * 2 * NB
                            nc.vector.tensor_copy(
                                out_t[0:125, c0:c0 + 2 * NB],
                                ihat[1:126, c0:c0 + 2 * NB].bitcast(dt.int8),
                            )
                        out_engs[k % 2].dma_start(
                            out=Y[k * L:(k + 1) * L, :], in_=out_t[0:125, :]
                        )
                        ihat = ihat_next
                        continue

                    o_pairs = []
                    for pair in range(2):
                        p0 = pair * 2 * NB
                        a2 = apool.tile([127, 2 * NB], dt.bfloat16, tag="A")
                        op = psO.tile([127, 2 * NB], dt.float32, tag="O")
                        d_ps = []
                        for half in range(2):
                            dp = psD.tile([127, NB], dt.float32, tag="D")
                            nc.tensor.matmul(
                                dp[:], wt["WD"][:],
                                ihat[:, p0 + half * NB:p0 + (half + 1) * NB],
                                start=True, stop=True,
                            )
                            d_ps.append(dp)
                        for half in range(2):
                            # abs zeroes rows 0-1 (WD cols 0-1 are zero)...
                            nc.scalar.activation(
                                a2[:, half * NB:(half + 1) * NB], d_ps[half][:],
                                func=mybir.ActivationFunctionType.Abs,
                                scale=float(C3),
                            )
                        # ...then the carry overwrites them (bf16, from the
                        # previous chunk's out_t rows 0-1; S0 for chunk 0).
                        # GpSimd: SBUF->SBUF is legal there and it keeps the
                        # carry copy off the two PSUM-drain queues.
                        if prev_out is None:
                            nc.sync.dma_start(
                                out=a2[0:2, :], in_=wd["S0"][:, p0:p0 + 2 * NB]
                            )
                        else:
                            nc.gpsimd.tensor_copy(
                                a2[0:2, :], prev_out[0:2, p0:p0 + 2 * NB]
                            )
                        for half in range(2):
                            c0 = p0 + half * NB
                            col = half * NB
                            nc.tensor.matmul(
                                op[:, col:col + NB], wt["W_IN"][:],
                                ihat[:, c0:c0 + NB], start=True, stop=False,
                            )
                            nc.tensor.matmul(
                                op[:, col:col + NB], wt["WA"][:],
                                a2[:, col:col + NB], start=False, stop=True,
                            )
                        o_pairs.append(op)
                    # PSUM -> SBUF with bias: ACT one quarter (it carries all
                    # four abs), DVE the rest
                    nc.scalar.activation(
                        out_t[:, 0:NB], o_pairs[0][:, 0:NB],
                        func=mybir.ActivationFunctionType.Identity,
                        bias=wt["BIAS"][:, 0:1],
                    )
                    nc.vector.tensor_scalar_add(
                        out_t[:, NB:2 * NB], o_pairs[0][:, NB:2 * NB],
                        wt["BIAS"][:, 0:1],
                    )
                    nc.vector.tensor_scalar_add(
                        out_t[:, 2 * NB:4 * NB], o_pairs[1][:, :],
                        wt["BIAS"][:, 0:1],
                    )
                    # emit chunk k-1's out-DMA here: its wait is satisfied by
                    # now, so the issuing sequencer (ACT for scalar) never
                    # blocks mid-chunk
                    if k > 0:
                        out_engs[(k - 1) % 2].dma_start(
                            out=Y[(k - 1) * L:k * L, :], in_=pend_out[2:127, :]
                        )
                    pend_out = out_t
                    prev_out = out_t
                    ihat = ihat_next
                out_engs[(NCH - 1) % 2].dma_start(
                    out=Y[(NCH - 1) * L:NCH * L, :], in_=pend_out[2:127, :]
                )

    nc.compile()
    return nc, w


_PROGRAM_CACHE = {}


def _get_program():
    if "nc" not in _PROGRAM_CACHE:
        nc, w = build_program()
        _PROGRAM_CACHE["nc"] = nc
        _PROGRAM_CACHE["w"] = w
    return _PROGRAM_CACHE["nc"], _PROGRAM_CACHE["w"]


def kernel(I_in: np.ndarray) -> np.ndarray:
    """Full-input entry point: I_in [8, 2000, 2048] fp32 -> out same shape."""
    from concourse.bass_utils import run_bass_kernel_spmd

    nc, w = _get_program()
    Xq = np.ascontiguousarray(I_in, dtype=np.float32).astype(X_DT)
    in_maps = [{"X": Xq[b], **w} for b in range(B)]
    last_err = None
    for _attempt in range(3):
        try:
            res = run_bass_kernel_spmd(nc, in_maps, list(range(B)))
            return np.stack(
                [res.results[b]["Y"].astype(np.float32) for b in range(B)], axis=0
            )
        except Exception as e:  # transient device errors: retry
            last_err = e
            import time as _time
            _time.sleep(5)
    raise last_err


if __name__ == "__main__":
    rng = np.random.default_rng(0)
    I = rng.standard_normal((B, T, N), dtype=np.float32)
    out = kernel(I)
    print(out.shape, out.dtype, np.abs(out).max())


# revision 22
# speedup vs baseline: 1.6241x; 1.4630x over previous
"""Trainium2 Bass kernel for CombinedSARAFilter.

Math: with D_t = I_t - I_{t-1} (I_{-1}=0), the module reduces to
    x_t = lam_r x_{t-1} + p D_t + q I_t
    o_t = lam_d o_{t-1} + a_d x_t + c3 |D_t|        (out = o, since TAU_RA == TAU_D)

Blocked linear scan, time chunks of L=125 on partitions; the 2e-2 tolerance
lets I/O compress: fp8(e4m3) input feeds the matmuls directly, bf16 output.
Per 512-lane block: 3 matmuls — WD (differences, for the |D| nonlinearity),
W_IN (input response, fp8), WA_AUG (abs response + carry response, bf16).
The (x, o-MU) carry rides rows 0-1 of the bf16 a-tile: the abs activation
zeroes them (WD's cols 0-1 are zero), then a cheap SBUF copy from the
previous chunk's bf16 out_t overwrites them — emitted after the abs so the
write order is right. DMA: input on sync, output alternating scalar/gpsimd
(per-queue DMA caps at ~120 GB/s, so each stream gets its own queue(s)).
"""
import sys

sys.path.insert(0, "/opt/trn_rl_repo")

import numpy as np
import ml_dtypes

BF16 = ml_dtypes.bfloat16
FP8 = ml_dtypes.float8_e4m3
X_DT = FP8

# filter constants
DT = 0.1
TAU_RA, K3 = 30.0, 2.0
TAU_R, TAU_D, K1, K2 = 5.0, 30.0, 0.05, 3.0
A_R = DT / TAU_R
A_D = DT / TAU_D
LAM_R = 1.0 - A_R
LAM_D = 1.0 - A_D
P = A_R * K2 / DT
Q = A_R * K1
C3 = K3 / TAU_RA
MU = 22.5  # mean shift: carry o-MU so the bf16 state stays small

B, T, N = 8, 2000, 2048
L = 125            # time chunk (on partitions)
NCH = T // L       # 16
NB = 512           # lane block (PSUM bank = 512 fp32)
NBLK = N // NB     # 4


def build_weights():
    """Host-side fp64 construction of the chunk filter matrices."""
    i = np.arange(L)
    Mr = np.tril(LAM_R ** (i[:, None] - i[None, :]))
    Md = np.tril(LAM_D ** (i[:, None] - i[None, :]))
    Bp = np.zeros((L, L + 1))
    Bp[i, i + 1] = 1.0
    Bp[i, i] = -1.0
    U = P * Bp
    U[:, 1:] += Q * np.eye(L)
    F1 = A_D * Md @ Mr @ U                  # [125 out, 126 in] o response to ihat
    v1 = LAM_D ** (i + 1)                   # o response to o'_in
    v2 = A_D * (Md @ (LAM_R ** (i + 1)))    # o response to x_in
    ones_resp = -A_D * MU * Md.sum(1)       # response of the constant -a_d*MU
    xrow_I = (Mr @ U)[L - 1]

    # out cols everywhere: 0=x_out, 1=o'_out, 2..126 = y_0..y_124

    # W_IN (lhsT) [126, 127] fp8: response to ihat rows [I_prev, I_0..I_124]
    W_in = np.zeros((126, 127))
    W_in[:, 0] = xrow_I
    W_in[:, 1] = F1[L - 1]
    W_in[:, 2:] = F1.T

    # WD (lhsT) [126, 127]: cols 0-1 zero (carry slots), col 2+t = D_t
    WD = np.zeros((126, 127))
    WD[i + 1, i + 2] = 1.0
    WD[i, i + 2] -= 1.0

    # WA_AUG (lhsT) [127, 127] bf16: rows 0-1 = carry response, rows 2.. = Md
    WA = np.zeros((127, 127))
    WA[0, 0] = LAM_R ** L
    WA[0, 1] = v2[L - 1]
    WA[0, 2:] = v2
    WA[1, 1] = v1[L - 1]
    WA[1, 2:] = v1
    WA[2:, 1] = Md[L - 1]
    WA[2:, 2:] = Md.T

    # bias column: x_out 0, o'_out ones_resp[-1], y rows ones_resp + MU
    bias = np.zeros((127, 1))
    bias[1, 0] = ones_resp[L - 1]
    bias[2:, 0] = ones_resp + MU

    s0 = np.zeros((2, N))
    s0[1] = -MU                      # initial carry (x=0, o'=-MU)
    z8 = np.zeros((1, N))            # chunk-0 I_prev row

    return {
        "W_IN": W_in.astype(FP8),
        "WD": WD.astype(FP8),
        "WA": WA.astype(BF16),
        "BIAS": bias.astype(np.float32),
        "S0": s0.astype(BF16),
        "Z8": z8.astype(FP8),
    }


_WDT = {"W_IN": "float8e4", "WD": "float8e4", "WA": "bfloat16",
        "BIAS": "float32", "S0": "bfloat16", "Z8": "float8e4"}


def build_program(reps: int = 1, mode: str = "full"):
    """Emit the single-core SPMD program. Returns (nc, weight_arrays)."""
    from concourse import bacc, mybir, tile

    dt = mybir.dt
    w = build_weights()

    nc = bacc.Bacc("TRN2", target_bir_lowering=False, debug=False)

    X = nc.dram_tensor("X", [T, N], dt.float8e4, kind="ExternalInput")
    Y = nc.dram_tensor("Y", [T, N], dt.bfloat16, kind="ExternalOutput")
    wd = {
        name: nc.dram_tensor(name, list(arr.shape), getattr(dt, _WDT[name]),
                             kind="ExternalInput")
        for name, arr in w.items()
    }

    with tile.TileContext(nc) as tc:
        with (
            tc.tile_pool(name="wpool", bufs=1) as wpool,
            tc.tile_pool(name="ipool", bufs=3) as ipool,
            tc.tile_pool(name="opool", bufs=3) as opool,
            tc.tile_pool(name="apool", bufs=4) as apool,
            tc.tile_pool(name="psD", bufs=2, space="PSUM") as psD,
            tc.tile_pool(name="psO", bufs=3, space="PSUM") as psO,
        ):
            wt = {}
            for name, arr in w.items():
                t_ = wpool.tile(list(arr.shape), getattr(dt, _WDT[name]), tag=name)
                nc.sync.dma_start(out=t_[:], in_=wd[name][:])
                wt[name] = t_

            out_engs = [nc.scalar, nc.gpsimd]
            for rep in range(reps):
                ihat = ipool.tile([126, N], dt.float8e4, tag="ihat")
                nc.sync.dma_start(out=ihat[0:1, :], in_=wd["Z8"][:])
                nc.sync.dma_start(out=ihat[1:126, :], in_=X[0:L, :])
                prev_out = None

                for k in range(NCH):
                    if k + 1 < NCH:
                        ihat_next = ipool.tile([126, N], dt.float8e4, tag="ihat")
                        nc.sync.dma_start(
                            out=ihat_next[:, :],
                            in_=X[(k + 1) * L - 1:(k + 2) * L, :],
                        )
                    else:
                        ihat_next = None

                    out_t = opool.tile([127, N], dt.bfloat16, tag="out")
                    if mode == "dma":
                        for half in range(2):
                            c0 = half * 2 * NB
                            nc.vector.tensor_copy(
                                out_t[0:125, c0:c0 + 2 * NB],
                                ihat[1:126, c0:c0 + 2 * NB].bitcast(dt.int8),
                            )
                        out_engs[k % 2].dma_start(
                            out=Y[k * L:(k + 1) * L, :], in_=out_t[0:125, :]
                        )
                        ihat = ihat_next
                        continue

                    o_pairs = []
                    for pair in range(2):
                        p0 = pair * 2 * NB
                        a2 = apool.tile([127, 2 * NB], dt.bfloat16, tag="A")
                        op = psO.tile([127, 2 * NB], dt.float32, tag="O")
                        d_ps = []
                        for half in range(2):
                            dp = psD.tile([127, NB], dt.float32, tag="D")
                            nc.tensor.matmul(
                                dp[:], wt["WD"][:],
                                ihat[:, p0 + half * NB:p0 + (half + 1) * NB],
                                start=True, stop=True,
                            )
                            d_ps.append(dp)
                        for half in range(2):
                            # abs zeroes rows 0-1 (WD cols 0-1 are zero)...
                            nc.scalar.activation(
                                a2[:, half * NB:(half + 1) * NB], d_ps[half][:],
                                func=mybir.ActivationFunctionType.Abs,
                                scale=float(C3),
                            )
                        # ...then the carry overwrites them (bf16, from the
                        # previous chunk's out_t rows 0-1; S0 for chunk 0).
                        # (DVE: GpSimd's Q7 software loop is ~5x slower for
                        # 2-partition copies, and it cannot touch PSUM at all.)
                        if prev_out is None:
                            nc.sync.dma_start(
                                out=a2[0:2, :], in_=wd["S0"][:, p0:p0 + 2 * NB]
                            )
                        else:
                            nc.vector.tensor_copy(
                                a2[0:2, :], prev_out[0:2, p0:p0 + 2 * NB]
                            )
                        for half in range(2):
                            c0 = p0 + half * NB
                            col = half * NB
                            nc.tensor.matmul(
                                op[:, col:col + NB], wt["W_IN"][:],
                                ihat[:, c0:c0 + NB], start=True, stop=False,
                            )
                            nc.tensor.matmul(
                                op[:, col:col + NB], wt["WA"][:],
                                a2[:, col:col + NB], start=False, stop=True,
                            )
                        o_pairs.append(op)
                    # PSUM -> SBUF with bias: ACT one quarter (it carries all
                    # four abs), DVE the rest
                    nc.scalar.activation(
                        out_t[:, 0:NB], o_pairs[0][:, 0:NB],
                        func=mybir.ActivationFunctionType.Identity,
                        bias=wt["BIAS"][:, 0:1],
                    )
                    nc.vector.tensor_scalar_add(
                        out_t[:, NB:2 * NB], o_pairs[0][:, NB:2 * NB],
                        wt["BIAS"][:, 0:1],
                    )
                    nc.vector.tensor_scalar_add(
                        out_t[:, 2 * NB:4 * NB], o_pairs[1][:, :],
                        wt["BIAS"][:, 0:1],
                    )
                    # emit chunk k-1's out-DMA here: its wait is satisfied by
                    # now, so the issuing sequencer (ACT for scalar) never
                    # blocks mid-chunk
                    if k > 0:
                        out_engs[(k - 1) % 2].dma_start(
                            out=Y[(k - 1) * L:k * L, :], in_=pend_out[2:127, :]
                        )
                    pend_out = out_t
                    prev_out = out_t
                    ihat = ihat_next
                out_engs[(NCH - 1) % 2].dma_start(
                    out=Y[(NCH - 1) * L:NCH * L, :], in_=pend_out[2:127, :]
                )

    nc.compile()
    return nc, w


_PROGRAM_CACHE = {}


def _get_program():
    if "nc" not in _PROGRAM_CACHE:
        nc, w = build_program()
        _PROGRAM_CACHE["nc"] = nc
        _PROGRAM_CACHE["w"] = w
    return _PROGRAM_CACHE["nc"], _PROGRAM_CACHE["w"]


def kernel(I_in: np.ndarray) -> np.ndarray:
    """Full-input entry point: I_in [8, 2000, 2048] fp32 -> out same shape."""
    from concourse.bass_utils import run_bass_kernel_spmd

    nc, w = _get_program()
    Xq = np.ascontiguousarray(I_in, dtype=np.float32).astype(X_DT)
    in_maps = [{"X": Xq[b], **w} for b in range(B)]
    last_err = None
    for _attempt in range(3):
        try:
            res = run_bass_kernel_spmd(nc, in_maps, list(range(B)))
            return np.stack(
                [res.results[b]["Y"].astype(np.float32) for b in range(B)], axis=0
            )
        except Exception as e:  # transient device errors: retry
            last_err = e
            import time as _time
            _time.sleep(5)
    raise last_err


if __name__ == "__main__":
    rng = np.random.default_rng(0)
    I = rng.standard_normal((B, T, N), dtype=np.float32)
    out = kernel(I)
    print(out.shape, out.dtype, np.abs(out).max())
